# revision 48
# baseline (speedup 1.0000x reference)
"""Trainium2 Bass kernel for nn_ModelRQuery_5806795784426.

Strategy (data-parallel over bags, 8 cores x 64 bags):
  - node_weight (cosine-sim softmax) is computed with the exact same eager
    jax ops as the reference, so the Huffman merge schedule derived from it
    is bit-faithful to the reference's argmin decisions on this backend.
  - The Huffman weight evolution is replayed on host (pure IEEE f32 adds on
    identical bits -> identical schedule), producing per-bag merge pairs.
  - Per bag, the 63 merges are list-scheduled into R pair-rounds (2
    independent merges per round -> M=128 output partitions, full PE array)
    with children >= 2 rounds earlier, so every round's gather can be
    prefetched one round ahead.  Each bag's root merge is pinned to the
    final round, slot 0, so the scores read straight out of SBUF.
  - Every tree node is consumed exactly once, so tanh is applied at
    PRODUCTION: DRAM feats rows hold tanh'd bf16 features (leaves are
    host-pre-tanh'd), making gathered operands transpose-ready with no
    scalar work on the consumption path.
  - Device round: indirect-DMA gather of 2x128 tanh'd bf16 rows, PE
    transpose, cast fp8, FC1/FC2 as fp8 DoubleRow matmuls (K=256/pass,
    2x PE throughput), tanh, scatter.  Merge executed in round r slot s
    writes DRAM slot 64+2r+s of its bag.
  - Final: raw f32 root features are DMA'd out; the tiny scores matmul
    (B,1024)@(1024,53) + sigmoid run on host.
"""

import numpy as np

NB = 64      # bags per core
NN = 64      # nodes (leaves) per bag
D = 1024
NSTEP = NN - 1
CPAD = 64    # rel classes padded 53 -> 64
NCORES = 8

_PROG = {}


def _build_program(R, SL, sync_rounds, zero_bias, use_fp8=True):
    key = (R, SL, tuple(sorted(sync_rounds)), zero_bias, use_fp8)
    if key in _PROG:
        return _PROG[key]
    import concourse.bass as bass
    import concourse.bacc as bacc
    import concourse.tile as tile

    mybir = bass.mybir
    f32 = mybir.dt.float32
    bf16 = mybir.dt.bfloat16
    f8 = mybir.dt.float8e4
    i32 = mybir.dt.int32
    TANH = mybir.ActivationFunctionType.Tanh
    SIG = mybir.ActivationFunctionType.Sigmoid
    COPY = mybir.ActivationFunctionType.Copy
    ADD = mybir.AluOpType.add
    DR = mybir.MatmulPerfMode.DoubleRow
    mmdt = f8 if use_fp8 else bf16

    nc = bacc.Bacc(None, target_bir_lowering=False)
    # feats rows: bag*SL + slot, tanh'd bf16; slots 0..63 = leaves
    # (host-prefilled tanh(rep)), slot 64+2r+s = tanh(output) of round r
    # in-round slot s.
    feats_d = nc.dram_tensor("feats", [NB * SL, D], bf16, kind="ExternalInput")
    w1t_d = nc.dram_tensor("w1t", [2 * D, D], mmdt, kind="ExternalInput")
    w2t_d = nc.dram_tensor("w2t", [D, D], mmdt, kind="ExternalInput")
    b1b_d = nc.dram_tensor("b1b", [128, D], f32, kind="ExternalInput")
    b2b_d = nc.dram_tensor("b2b", [128, D], f32, kind="ExternalInput")
    gidx_d = nc.dram_tensor("gidx", [128, 2 * R], i32, kind="ExternalInput")
    ident_d = nc.dram_tensor("ident", [128, 128], bf16, kind="ExternalInput")
    out_d = nc.dram_tensor("out", [NB, D], f32, kind="ExternalOutput")

    with tile.TileContext(nc) as tc:
        with tc.tile_pool(name="const", bufs=1) as cp, \
             tc.tile_pool(name="work", bufs=2) as wp, \
             tc.tile_pool(name="gat", bufs=3) as xp, \
             tc.tile_pool(name="tpp", bufs=2, space="PSUM") as pt, \
             tc.tile_pool(name="mmp", bufs=1, space="PSUM") as pm:

            feats3 = feats_d[:].rearrange("(b s) d -> b s d", s=SL)

            # offsets + identity first so the first gather & transposes can
            # start while the weight DMAs stream in
            gixs = cp.tile([128, 2 * R], i32)
            nc.sync.dma_start(out=gixs[:], in_=gidx_d[:])
            ident = cp.tile([128, 128], bf16)

            def emit_gather(r):
                x = xp.tile([128, 2 * D], bf16, tag="x")
                nc.gpsimd.indirect_dma_start(
                    out=x[:, 0:D], out_offset=None, in_=feats_d[:],
                    in_offset=bass.IndirectOffsetOnAxis(ap=gixs[:, 2 * r:2 * r + 1], axis=0))
                nc.gpsimd.indirect_dma_start(
                    out=x[:, D:2 * D], out_offset=None, in_=feats_d[:],
                    in_offset=bass.IndirectOffsetOnAxis(ap=gixs[:, 2 * r + 1:2 * r + 2], axis=0))
                return x

            def emit_transpose_x_half(xt, h):
                # 8 PE transposes (128x128 bf16) -> one 1-bank PSUM tile
                ps = pt.tile([128, 8, 128], bf16, tag="xtp")
                for j in range(8):
                    c = 8 * h + j
                    nc.tensor.transpose(out=ps[:, j, :], in_=xt[:, 128 * c:128 * (c + 1)],
                                        identity=ident[:])
                return ps

            def emit_transpose_x(xt):
                # fallback: full transpose + casts back-to-back (sync rounds)
                xT = wp.tile([128, 16, 128], mmdt, tag="xT")
                for h in range(2):
                    ps = emit_transpose_x_half(xt, h)
                    nc.vector.tensor_copy(out=xT[:, 8 * h:8 * h + 8, :], in_=ps[:])
                return xT

            xg = {0: emit_gather(0)}
            nc.sync.dma_start(out=ident[:], in_=ident_d[:])

            w1t = cp.tile([128, 8, 2, D], mmdt)
            nc.sync.dma_start(out=w1t[:], in_=w1t_d[:].rearrange("(c two p) d -> p c two d", two=2, p=128))
            w2t = cp.tile([128, 4, 2, D], mmdt)
            nc.sync.dma_start(out=w2t[:], in_=w2t_d[:].rearrange("(c two p) d -> p c two d", two=2, p=128))
            if not zero_bias:
                b1b = cp.tile([128, D], f32)
                nc.sync.dma_start(out=b1b[:], in_=b1b_d[:])
                b2b = cp.tile([128, D], f32)
                nc.sync.dma_start(out=b2b[:], in_=b2b_d[:])

            xTd = {0: emit_transpose_x(xg.pop(0))}
            for r in range(R):
                last = (r == R - 1)
                xT = xTd.pop(r)
                nxt = r + 1
                # prefetch gather(r+1): emitted after scatter(r-1), so it
                # only waits on scatter(r-1) -> safe iff round r+1's children
                # are in rounds <= r-1 (the packer guarantees <= r-2)
                if nxt < R and nxt not in sync_rounds:
                    xg[nxt] = emit_gather(nxt)

                # FC1: h[(s,b), :] = x @ W1  (x already tanh'd; K=2048)
                h0 = pm.tile([128, 512], f32, tag="h0")
                h1 = pm.tile([128, 512], f32, tag="h1")
                for hn, ht in ((1, h1), (0, h0)):
                    if use_fp8:
                        for c in range(8):
                            nc.tensor.matmul(ht[:], xT[:, 2 * c:2 * c + 2, :],
                                             w1t[:, c, :, 512 * hn:512 * (hn + 1)],
                                             start=(c == 0), stop=(c == 7), perf_mode=DR)
                    else:
                        for c in range(8):
                            for i in range(2):
                                nc.tensor.matmul(ht[:], xT[:, 2 * c + i, :],
                                                 w1t[:, c, i, 512 * hn:512 * (hn + 1)],
                                                 start=(c == 0 and i == 0), stop=(c == 7 and i == 1))

                htt = wp.tile([128, D], bf16, tag="htt")
                if zero_bias:
                    nc.scalar.activation(out=htt[:, 512:1024], in_=h1[:], func=TANH)
                    nc.scalar.activation(out=htt[:, 0:512], in_=h0[:], func=TANH)
                else:
                    hbt = wp.tile([128, D], f32, tag="hbt")
                    nc.vector.tensor_tensor(out=hbt[:, 0:512], in0=h0[:], in1=b1b[:, 0:512], op=ADD)
                    nc.vector.tensor_tensor(out=hbt[:, 512:1024], in0=h1[:], in1=b1b[:, 512:1024], op=ADD)
                    nc.scalar.activation(out=htt[:, 0:512], in_=hbt[:, 0:512], func=TANH)
                    nc.scalar.activation(out=htt[:, 512:1024], in_=hbt[:, 512:1024], func=TANH)

                # h transpose quarters interleaved with next round's X
                # transposes: the PE covers the tanh-h1/cast latency and the
                # vector queue stays [hT q0, hT q1, XT h0, XT h1] so FC2
                # never waits on a cast stuck behind XT work.
                pipelined = nxt < R and nxt not in sync_rounds
                xnext = xg.pop(nxt) if pipelined else None
                hT = wp.tile([128, 8, 128], mmdt, tag="hT")
                for q in (1, 0):
                    ps = pt.tile([128, 4, 128], bf16, tag="htp")
                    for j in range(4):
                        c = 4 * q + j
                        nc.tensor.transpose(out=ps[:, j, :], in_=htt[:, 128 * c:128 * (c + 1)],
                                            identity=ident[:])
                    nc.vector.tensor_copy(out=hT[:, 4 * q:4 * q + 4, :], in_=ps[:])
                    if pipelined and q == 1:
                        xps0 = emit_transpose_x_half(xnext, 0)
                if pipelined:
                    xps1 = emit_transpose_x_half(xnext, 1)
                    xTn = wp.tile([128, 16, 128], mmdt, tag="xT")
                    # XT casts go on the scalar engine: the vector queue must
                    # stay short so FC2 gets hT quickly; scalar's tanh-f can
                    # afford the delay
                    nc.scalar.activation(out=xTn[:, 0:8, :], in_=xps0[:], func=COPY)
                    nc.scalar.activation(out=xTn[:, 8:16, :], in_=xps1[:], func=COPY)
                    xTd[nxt] = xTn

                # FC2; each half's tanh is emitted right after that
                # half's accumulation stops, so tanh-f0 overlaps the f1
                # matmuls and the (full-row) scatter launches earlier --
                # that shortens the scatter -> next-next-round-gather chain
                f0 = pm.tile([128, 512], f32, tag="f0")
                f1 = pm.tile([128, 512], f32, tag="f1")
                ftb = None if last else wp.tile([128, D], bf16, tag="ftb")
                fbt = None if (last or zero_bias) else wp.tile([128, D], f32, tag="fbt")
                for fn, ft in ((0, f0), (1, f1)):
                    if use_fp8:
                        for ci, c in enumerate((2, 3, 0, 1)):
                            nc.tensor.matmul(ft[:], hT[:, 2 * c:2 * c + 2, :],
                                             w2t[:, c, :, 512 * fn:512 * (fn + 1)],
                                             start=(ci == 0), stop=(ci == 3), perf_mode=DR)
                    else:
                        for c in range(4):
                            for i in range(2):
                                nc.tensor.matmul(ft[:], hT[:, 2 * c + i, :],
                                                 w2t[:, c, i, 512 * fn:512 * (fn + 1)],
                                                 start=(c == 0 and i == 0), stop=(c == 3 and i == 1))
                    if not last:
                        cs = slice(512 * fn, 512 * (fn + 1))
                        if zero_bias:
                            nc.scalar.activation(out=ftb[:, cs], in_=ft[:], func=TANH)
                        else:
                            nc.vector.tensor_tensor(out=fbt[:, cs], in0=ft[:], in1=b2b[:, cs], op=ADD)
                            nc.scalar.activation(out=ftb[:, cs], in_=fbt[:, cs], func=TANH)

                if last:
                    # roots are partitions 0:64 of f0/f1; ship the raw f32
                    # root features out -- the (B,1024)@(1024,53) scores +
                    # sigmoid are done on host (0.4% of FLOPs)
                    rootf = wp.tile([64, D], f32, tag="rootf")
                    if zero_bias:
                        nc.vector.tensor_copy(out=rootf[:, 0:512], in_=f0[0:64, :])
                        nc.vector.tensor_copy(out=rootf[:, 512:1024], in_=f1[0:64, :])
                    else:
                        nc.vector.tensor_tensor(out=rootf[:, 0:512], in0=f0[0:64, :], in1=b2b[0:64, 0:512], op=ADD)
                        nc.vector.tensor_tensor(out=rootf[:, 512:1024], in0=f1[0:64, :], in1=b2b[0:64, 512:1024], op=ADD)
                    nc.sync.dma_start(out=out_d[:], in_=rootf[:])
                else:
                    for s in range(2):
                        nc.sync.dma_start(out=feats3[:, 64 + 2 * r + s, :],
                                          in_=ftb[64 * s:64 * (s + 1), :])
                    # sync-round gather + transposes (trail this scatter)
                    if nxt < R and nxt in sync_rounds:
                        xg[nxt] = emit_gather(nxt)
                        xTd[nxt] = emit_transpose_x(xg.pop(nxt))

    nc.compile()
    _PROG[key] = nc
    return nc


def _node_weight_like_reference(rep, n_per_bag):
    """Bit-faithful mirror of the reference's eager node_weight computation
    (reference runs on CPU jax; mirror that exactly)."""
    import jax
    import jax.numpy as jnp
    cpu = jax.local_devices(backend="cpu")[0]
    with jax.default_device(cpu):
        d = rep.shape[-1]
        bags = jnp.asarray(np.ascontiguousarray(rep, dtype=np.float32)).reshape(-1, n_per_bag, d)
        norms = jnp.linalg.norm(bags, axis=-1)
        gram = jnp.einsum('bnd,bmd->bnm', bags, bags)
        sims = gram / jnp.maximum(norms[:, :, None] * norms[:, None, :], 1e-8)
        node_distance = sims.sum(axis=1)
        node_weight = jax.nn.softmax(node_distance, axis=-1)
        return np.asarray(node_weight).astype(np.float32)


def _huffman_schedule(w):
    """Replay the reference scan's weight bookkeeping (exact f32) and emit
    per-bag merge operand slots: leaves 0..63, merge t -> 64+t."""
    B, n = w.shape
    wref = w.copy()
    alive = np.ones((B, n), bool)
    prov = np.tile(np.arange(n, dtype=np.int64), (B, 1))
    ar = np.arange(B)
    gl = np.zeros((B, n - 1), np.int64)
    gr = np.zeros((B, n - 1), np.int64)
    INF = np.float32(np.inf)
    for t in range(n - 1):
        wm = np.where(alive, wref, INF)
        i1 = np.argmin(wm, axis=1)
        wm2 = wm.copy()
        wm2[ar, i1] = INF
        i2 = np.argmin(wm2, axis=1)
        gl[:, t] = prov[ar, i1]
        gr[:, t] = prov[ar, i2]
        wref[ar, i1] = wm[ar, i1] + wm[ar, i2]
        alive[ar, i2] = False
        prov[ar, i1] = n + t
    return gl, gr


def _pack_rounds(gl, gr, n=NN, dist=2):
    """List-schedule each bag's n-1 merges into pair-rounds (2 independent
    merges per round; children must be done <= r-dist; priority = longest
    path to root).  The root merge is then pinned to (last real round + 1,
    slot 0) for every bag, so the device reads all roots from the final
    round's SBUF result tile.  Returns (rounds_of, slot_of, R, sync_rounds).
    sync_rounds lists rounds with a child at distance 1 (gather must trail
    the previous round's scatter)."""
    B, m = gl.shape
    rounds_of = np.zeros((B, m), np.int64)
    slot_of = np.zeros((B, m), np.int64)
    last_nonroot = 0
    root_child_max = 0
    for b in range(B):
        cl, cr = gl[b], gr[b]
        parents = np.full(m, -1, np.int64)
        ndep = np.zeros(m, np.int32)
        for j in range(m):
            for s in (cl[j], cr[j]):
                if s >= n:
                    ndep[j] += 1
                    parents[s - n] = j
        height = np.zeros(m, np.int64)
        for j in range(m - 1, -1, -1):
            p = parents[j]
            if p >= 0:
                height[j] = height[p] + 1
        done = np.full(m, 10**9, np.int64)
        remaining = ndep.copy()
        scheduled = 0
        r = 0
        while scheduled < m:
            ready = [j for j in range(m)
                     if remaining[j] == 0 and done[j] == 10**9
                     and all((s < n or done[s - n] <= r - dist) for s in (cl[j], cr[j]))]
            ready.sort(key=lambda j: (-height[j], j))
            for s_idx, j in enumerate(ready[:2]):
                rounds_of[b, j] = r
                slot_of[b, j] = s_idx
                done[j] = r
                scheduled += 1
                p = parents[j]
                if p >= 0:
                    remaining[p] -= 1
            r += 1
            assert r < 4 * m, "packer stuck"
        last_nonroot = max(last_nonroot, rounds_of[b, :m - 1].max())
        for s in (cl[m - 1], cr[m - 1]):
            if s >= n:
                root_child_max = max(root_child_max, int(rounds_of[b, s - n]))
    # root round: after every non-root merge AND >= 2 past every root child
    # so the root round's gather is prefetchable (non-sync)
    root_round = max(last_nonroot + 1, root_child_max + 2)
    rounds_of[:, m - 1] = root_round
    slot_of[:, m - 1] = 0
    R = root_round + 1
    sync_rounds = set()
    for b in range(B):
        for j in range(m):
            r = rounds_of[b, j]
            for s in (gl[b, j], gr[b, j]):
                if s >= n and rounds_of[b, s - n] > r - 2:
                    assert rounds_of[b, s - n] <= r - 1, "child not strictly earlier"
                    sync_rounds.add(int(r))
    return rounds_of, slot_of, R, sync_rounds


def _prepare(rep, fc1_w, fc1_b, fc2_w, fc2_b, rel_emb, n_per_bag, **kw):
    n_per_bag = int(n_per_bag)
    assert n_per_bag == NN and rep.shape[-1] == D
    rep = np.ascontiguousarray(rep, dtype=np.float32)

    w = _node_weight_like_reference(rep, n_per_bag)
    gl, gr = _huffman_schedule(w)
    rounds_of, slot_of, R, sync_rounds = _pack_rounds(gl, gr)
    SL = 64 + 2 * R
    zb = (not np.any(np.asarray(fc1_b))) and (not np.any(np.asarray(fc2_b)))
    use_fp8 = True
    nc = _build_program(R, SL, frozenset(sync_rounds), zb, use_fp8)

    import ml_dtypes
    mmdt = ml_dtypes.float8_e4m3fn if use_fp8 else ml_dtypes.bfloat16
    w1t = np.ascontiguousarray(np.asarray(fc1_w, np.float32).T).astype(mmdt)   # (2D, D)
    w2t = np.ascontiguousarray(np.asarray(fc2_w, np.float32).T).astype(mmdt)   # (D, D)
    b1b = np.ascontiguousarray(np.broadcast_to(np.asarray(fc1_b, np.float32), (128, D)))
    b2b = np.ascontiguousarray(np.broadcast_to(np.asarray(fc2_b, np.float32), (128, D)))
    ident = np.eye(128, dtype=ml_dtypes.bfloat16)

    m = gl.shape[1]
    merge_slot = 64 + 2 * rounds_of + slot_of          # (B, 63)

    in_maps = []
    for c in range(NCORES):
        b0 = c * NB
        gidx = np.zeros((128, 2 * R), np.int32)
        for s in range(2):
            for lb in range(NB):
                gidx[s * NB + lb, :] = lb * SL   # pads read a leaf row
        for lb in range(NB):
            b = b0 + lb
            for j in range(m):
                r = rounds_of[b, j]
                s = slot_of[b, j]
                ls, rs = gl[b, j], gr[b, j]
                ls = ls if ls < NN else merge_slot[b, ls - NN]
                rs = rs if rs < NN else merge_slot[b, rs - NN]
                gidx[s * NB + lb, 2 * r] = lb * SL + ls
                gidx[s * NB + lb, 2 * r + 1] = lb * SL + rs

        feats = np.zeros((NB * SL, D), ml_dtypes.bfloat16)
        feats.reshape(NB, SL, D)[:, 0:NN, :] = np.tanh(
            rep[b0 * NN:(b0 + NB) * NN].reshape(NB, NN, D)).astype(ml_dtypes.bfloat16)
        in_maps.append({
            "feats": feats,
            "w1t": w1t, "w2t": w2t,
            "b1b": b1b, "b2b": b2b, "gidx": gidx, "ident": ident,
        })
    return nc, in_maps


def kernel(rep, fc1_w, fc1_b, fc2_w, fc2_b, rel_emb, n_per_bag, **kw):
    nc, in_maps = _prepare(rep, fc1_w, fc1_b, fc2_w, fc2_b, rel_emb, n_per_bag)
    from concourse import bass_utils
    res = bass_utils.run_bass_kernel_spmd(nc, in_maps, core_ids=list(range(NCORES)))
    root = np.concatenate([res.results[c]["out"] for c in range(NCORES)], axis=0)
    scores = root.astype(np.float32) @ np.asarray(rel_emb, np.float32).T
    out = 1.0 / (1.0 + np.exp(-scores, dtype=np.float64))
    return np.ascontiguousarray(out.astype(np.float32))


# revision 49
# speedup vs baseline: 1.0124x; 1.0124x over previous
"""Trainium2 Bass kernel for nn_ModelRQuery_5806795784426.

Strategy (data-parallel over bags, 8 cores x 64 bags):
  - node_weight (cosine-sim softmax) is computed with the exact same eager
    jax ops as the reference, so the Huffman merge schedule derived from it
    is bit-faithful to the reference's argmin decisions on this backend.
  - The Huffman weight evolution is replayed on host (pure IEEE f32 adds on
    identical bits -> identical schedule), producing per-bag merge pairs.
  - Per bag, the 63 merges are list-scheduled into R pair-rounds (2
    independent merges per round -> M=128 output partitions, full PE array)
    with children >= 2 rounds earlier, so every round's gather can be
    prefetched one round ahead.  Each bag's root merge is pinned to the
    final round, slot 0, so the scores read straight out of SBUF.
  - Every tree node is consumed exactly once, so tanh is applied at
    PRODUCTION: DRAM feats rows hold tanh'd bf16 features (leaves are
    host-pre-tanh'd), making gathered operands transpose-ready with no
    scalar work on the consumption path.
  - Device round: indirect-DMA gather of 2x128 tanh'd bf16 rows, PE
    transpose, cast fp8, FC1/FC2 as fp8 DoubleRow matmuls (K=256/pass,
    2x PE throughput), tanh, scatter.  Merge executed in round r slot s
    writes DRAM slot 64+2r+s of its bag.
  - Final: raw f32 root features are DMA'd out; the tiny scores matmul
    (B,1024)@(1024,53) + sigmoid run on host.
"""

import numpy as np

NB = 64      # bags per core
NN = 64      # nodes (leaves) per bag
D = 1024
NSTEP = NN - 1
CPAD = 64    # rel classes padded 53 -> 64
NCORES = 8

_PROG = {}


def _build_program(R, SL, sync_rounds, zero_bias, use_fp8=True, direct01=False):
    key = (R, SL, tuple(sorted(sync_rounds)), zero_bias, use_fp8, direct01, "v18")
    if key in _PROG:
        return _PROG[key]
    import concourse.bass as bass
    import concourse.bacc as bacc
    import concourse.tile as tile

    mybir = bass.mybir
    f32 = mybir.dt.float32
    bf16 = mybir.dt.bfloat16
    f8 = mybir.dt.float8e4
    i32 = mybir.dt.int32
    TANH = mybir.ActivationFunctionType.Tanh
    SIG = mybir.ActivationFunctionType.Sigmoid
    COPY = mybir.ActivationFunctionType.Copy
    ADD = mybir.AluOpType.add
    DR = mybir.MatmulPerfMode.DoubleRow
    mmdt = f8 if use_fp8 else bf16

    nc = bacc.Bacc(None, target_bir_lowering=False)
    # feats rows: bag*SL + slot, tanh'd bf16; slots 0..63 = leaves
    # (host-prefilled tanh(rep)), slot 64+2r+s = tanh(output) of round r
    # in-round slot s.
    feats_d = nc.dram_tensor("feats", [NB * SL, D], bf16, kind="ExternalInput")
    w1t_d = nc.dram_tensor("w1t", [2 * D, D], mmdt, kind="ExternalInput")
    w2t_d = nc.dram_tensor("w2t", [D, D], mmdt, kind="ExternalInput")
    b1b_d = nc.dram_tensor("b1b", [128, D], f32, kind="ExternalInput")
    b2b_d = nc.dram_tensor("b2b", [128, D], f32, kind="ExternalInput")
    gidx_d = nc.dram_tensor("gidx", [128, 2 * R], i32, kind="ExternalInput")
    ident_d = nc.dram_tensor("ident", [128, 128], bf16, kind="ExternalInput")
    out_d = nc.dram_tensor("out", [NB, D], f32, kind="ExternalOutput")

    with tile.TileContext(nc) as tc:
        with tc.tile_pool(name="const", bufs=1) as cp, \
             tc.tile_pool(name="work", bufs=2) as wp, \
             tc.tile_pool(name="gat", bufs=3) as xp, \
             tc.tile_pool(name="tpp", bufs=2, space="PSUM") as pt, \
             tc.tile_pool(name="mmp", bufs=1, space="PSUM") as pm:

            feats3 = feats_d[:].rearrange("(b s) d -> b s d", s=SL)

            gixs = cp.tile([128, 2 * R], i32)
            ident = cp.tile([128, 128], bf16)

            def emit_gather(r):
                x = xp.tile([128, 2, D], bf16, tag="x")
                if direct01 and r <= 1:
                    # leaves are renumbered per bag so rounds 0/1 read fixed
                    # slots 4r .. 4r+3 -> plain direct DMAs, no DGE latency
                    base = 4 * r
                    nc.sync.dma_start(out=x[0:64, :, :], in_=feats3[:, base:base + 2, :])
                    nc.sync.dma_start(out=x[64:128, :, :], in_=feats3[:, base + 2:base + 4, :])
                else:
                    nc.gpsimd.indirect_dma_start(
                        out=x[:, 0, :], out_offset=None, in_=feats_d[:],
                        in_offset=bass.IndirectOffsetOnAxis(ap=gixs[:, 2 * r:2 * r + 1], axis=0))
                    nc.gpsimd.indirect_dma_start(
                        out=x[:, 1, :], out_offset=None, in_=feats_d[:],
                        in_offset=bass.IndirectOffsetOnAxis(ap=gixs[:, 2 * r + 1:2 * r + 2], axis=0))
                return x

            def emit_transpose_x_half(xt, h):
                # 8 PE transposes (128x128 bf16) -> one 1-bank PSUM tile
                # (operand h of the gathered pair)
                ps = pt.tile([128, 8, 128], bf16, tag="xtp")
                for j in range(8):
                    nc.tensor.transpose(out=ps[:, j, :], in_=xt[:, h, 128 * j:128 * (j + 1)],
                                        identity=ident[:])
                return ps

            def emit_transpose_x(xt):
                # fallback: full transpose + casts back-to-back (sync rounds)
                xT = wp.tile([128, 16, 128], mmdt, tag="xT")
                for h in range(2):
                    ps = emit_transpose_x_half(xt, h)
                    nc.vector.tensor_copy(out=xT[:, 8 * h:8 * h + 8, :], in_=ps[:])
                return xT

            xg = {0: emit_gather(0)}
            nc.sync.dma_start(out=ident[:], in_=ident_d[:])
            nc.sync.dma_start(out=gixs[:], in_=gidx_d[:])

            w1t = cp.tile([128, 8, 2, D], mmdt)
            nc.sync.dma_start(out=w1t[:], in_=w1t_d[:].rearrange("(c two p) d -> p c two d", two=2, p=128))
            w2t = cp.tile([128, 4, 2, D], mmdt)
            nc.sync.dma_start(out=w2t[:], in_=w2t_d[:].rearrange("(c two p) d -> p c two d", two=2, p=128))
            if not zero_bias:
                b1b = cp.tile([128, D], f32)
                nc.sync.dma_start(out=b1b[:], in_=b1b_d[:])
                b2b = cp.tile([128, D], f32)
                nc.sync.dma_start(out=b2b[:], in_=b2b_d[:])

            xTd = {0: emit_transpose_x(xg.pop(0))}
            for r in range(R):
                last = (r == R - 1)
                xT = xTd.pop(r)
                nxt = r + 1
                # prefetch gather(r+1): emitted after scatter(r-1), so it
                # only waits on scatter(r-1) -> safe iff round r+1's children
                # are in rounds <= r-1 (the packer guarantees <= r-2)
                if nxt < R and nxt not in sync_rounds:
                    xg[nxt] = emit_gather(nxt)

                # FC1: h[(s,b), :] = x @ W1  (x already tanh'd; K=2048)
                h0 = pm.tile([128, 512], f32, tag="h0")
                h1 = pm.tile([128, 512], f32, tag="h1")
                for hn, ht in ((1, h1), (0, h0)):
                    if use_fp8:
                        for c in range(8):
                            nc.tensor.matmul(ht[:], xT[:, 2 * c:2 * c + 2, :],
                                             w1t[:, c, :, 512 * hn:512 * (hn + 1)],
                                             start=(c == 0), stop=(c == 7), perf_mode=DR)
                    else:
                        for c in range(8):
                            for i in range(2):
                                nc.tensor.matmul(ht[:], xT[:, 2 * c + i, :],
                                                 w1t[:, c, i, 512 * hn:512 * (hn + 1)],
                                                 start=(c == 0 and i == 0), stop=(c == 7 and i == 1))

                htt = wp.tile([128, D], bf16, tag="htt")
                if zero_bias:
                    nc.scalar.activation(out=htt[:, 512:1024], in_=h1[:], func=TANH)
                    nc.scalar.activation(out=htt[:, 0:512], in_=h0[:], func=TANH)
                else:
                    hbt = wp.tile([128, D], f32, tag="hbt")
                    nc.vector.tensor_tensor(out=hbt[:, 0:512], in0=h0[:], in1=b1b[:, 0:512], op=ADD)
                    nc.vector.tensor_tensor(out=hbt[:, 512:1024], in0=h1[:], in1=b1b[:, 512:1024], op=ADD)
                    nc.scalar.activation(out=htt[:, 0:512], in_=hbt[:, 0:512], func=TANH)
                    nc.scalar.activation(out=htt[:, 512:1024], in_=hbt[:, 512:1024], func=TANH)

                # h transpose quarters interleaved with next round's X
                # transposes: the PE covers the tanh-h1/cast latency and the
                # vector queue stays [hT q0, hT q1, XT h0, XT h1] so FC2
                # never waits on a cast stuck behind XT work.
                pipelined = nxt < R and nxt not in sync_rounds
                xnext = xg.pop(nxt) if pipelined else None
                hT = wp.tile([128, 8, 128], mmdt, tag="hT")
                for q in (1, 0):
                    ps = pt.tile([128, 4, 128], bf16, tag="htp")
                    for j in range(4):
                        c = 4 * q + j
                        nc.tensor.transpose(out=ps[:, j, :], in_=htt[:, 128 * c:128 * (c + 1)],
                                            identity=ident[:])
                    nc.vector.tensor_copy(out=hT[:, 4 * q:4 * q + 4, :], in_=ps[:])
                    if pipelined and q == 1:
                        xps0 = emit_transpose_x_half(xnext, 0)
                if pipelined:
                    xps1 = emit_transpose_x_half(xnext, 1)
                    xTn = wp.tile([128, 16, 128], mmdt, tag="xT")
                    # XT casts go on the scalar engine: the vector queue must
                    # stay short so FC2 gets hT quickly; scalar's tanh-f can
                    # afford the delay
                    nc.scalar.activation(out=xTn[:, 0:8, :], in_=xps0[:], func=COPY)
                    nc.scalar.activation(out=xTn[:, 8:16, :], in_=xps1[:], func=COPY)
                    xTd[nxt] = xTn

                # FC2; each half's tanh is emitted right after that
                # half's accumulation stops, so tanh-f0 overlaps the f1
                # matmuls and the (full-row) scatter launches earlier --
                # that shortens the scatter -> next-next-round-gather chain
                f0 = pm.tile([128, 512], f32, tag="f0")
                f1 = pm.tile([128, 512], f32, tag="f1")
                ftb = None if last else wp.tile([128, D], bf16, tag="ftb")
                fbt = None if (last or zero_bias) else wp.tile([128, D], f32, tag="fbt")
                for fn, ft in ((0, f0), (1, f1)):
                    if use_fp8:
                        for ci, c in enumerate((2, 3, 0, 1)):
                            nc.tensor.matmul(ft[:], hT[:, 2 * c:2 * c + 2, :],
                                             w2t[:, c, :, 512 * fn:512 * (fn + 1)],
                                             start=(ci == 0), stop=(ci == 3), perf_mode=DR)
                    else:
                        for c in range(4):
                            for i in range(2):
                                nc.tensor.matmul(ft[:], hT[:, 2 * c + i, :],
                                                 w2t[:, c, i, 512 * fn:512 * (fn + 1)],
                                                 start=(c == 0 and i == 0), stop=(c == 3 and i == 1))
                    if not last:
                        cs = slice(512 * fn, 512 * (fn + 1))
                        if zero_bias:
                            nc.scalar.activation(out=ftb[:, cs], in_=ft[:], func=TANH)
                        else:
                            nc.vector.tensor_tensor(out=fbt[:, cs], in0=ft[:], in1=b2b[:, cs], op=ADD)
                            nc.scalar.activation(out=ftb[:, cs], in_=fbt[:, cs], func=TANH)

                if last:
                    # roots are partitions 0:64 of f0/f1; ship the raw f32
                    # root features out -- the (B,1024)@(1024,53) scores +
                    # sigmoid are done on host (0.4% of FLOPs)
                    rootf = wp.tile([64, D], f32, tag="rootf")
                    if zero_bias:
                        nc.vector.tensor_copy(out=rootf[:, 0:512], in_=f0[0:64, :])
                        nc.vector.tensor_copy(out=rootf[:, 512:1024], in_=f1[0:64, :])
                    else:
                        nc.vector.tensor_tensor(out=rootf[:, 0:512], in0=f0[0:64, :], in1=b2b[0:64, 0:512], op=ADD)
                        nc.vector.tensor_tensor(out=rootf[:, 512:1024], in0=f1[0:64, :], in1=b2b[0:64, 512:1024], op=ADD)
                    nc.sync.dma_start(out=out_d[:], in_=rootf[:])
                else:
                    for s in range(2):
                        nc.sync.dma_start(out=feats3[:, 64 + 2 * r + s, :],
                                          in_=ftb[64 * s:64 * (s + 1), :])
                    # sync-round gather + transposes (trail this scatter)
                    if nxt < R and nxt in sync_rounds:
                        xg[nxt] = emit_gather(nxt)
                        xTd[nxt] = emit_transpose_x(xg.pop(nxt))

    nc.compile()
    _PROG[key] = nc
    return nc


def _node_weight_like_reference(rep, n_per_bag):
    """Bit-faithful mirror of the reference's eager node_weight computation
    (reference runs on CPU jax; mirror that exactly)."""
    import jax
    import jax.numpy as jnp
    cpu = jax.local_devices(backend="cpu")[0]
    with jax.default_device(cpu):
        d = rep.shape[-1]
        bags = jnp.asarray(np.ascontiguousarray(rep, dtype=np.float32)).reshape(-1, n_per_bag, d)
        norms = jnp.linalg.norm(bags, axis=-1)
        gram = jnp.einsum('bnd,bmd->bnm', bags, bags)
        sims = gram / jnp.maximum(norms[:, :, None] * norms[:, None, :], 1e-8)
        node_distance = sims.sum(axis=1)
        node_weight = jax.nn.softmax(node_distance, axis=-1)
        return np.asarray(node_weight).astype(np.float32)


def _huffman_schedule(w):
    """Replay the reference scan's weight bookkeeping (exact f32) and emit
    per-bag merge operand slots: leaves 0..63, merge t -> 64+t."""
    B, n = w.shape
    wref = w.copy()
    alive = np.ones((B, n), bool)
    prov = np.tile(np.arange(n, dtype=np.int64), (B, 1))
    ar = np.arange(B)
    gl = np.zeros((B, n - 1), np.int64)
    gr = np.zeros((B, n - 1), np.int64)
    INF = np.float32(np.inf)
    for t in range(n - 1):
        wm = np.where(alive, wref, INF)
        i1 = np.argmin(wm, axis=1)
        wm2 = wm.copy()
        wm2[ar, i1] = INF
        i2 = np.argmin(wm2, axis=1)
        gl[:, t] = prov[ar, i1]
        gr[:, t] = prov[ar, i2]
        wref[ar, i1] = wm[ar, i1] + wm[ar, i2]
        alive[ar, i2] = False
        prov[ar, i1] = n + t
    return gl, gr


def _pack_rounds(gl, gr, n=NN, dist=2):
    """List-schedule each bag's n-1 merges into pair-rounds (2 independent
    merges per round; children must be done <= r-dist; priority = longest
    path to root).  The root merge is then pinned to (last real round + 1,
    slot 0) for every bag, so the device reads all roots from the final
    round's SBUF result tile.  Returns (rounds_of, slot_of, R, sync_rounds).
    sync_rounds lists rounds with a child at distance 1 (gather must trail
    the previous round's scatter)."""
    B, m = gl.shape
    rounds_of = np.zeros((B, m), np.int64)
    slot_of = np.zeros((B, m), np.int64)
    last_nonroot = 0
    root_child_max = 0
    for b in range(B):
        cl, cr = gl[b], gr[b]
        parents = np.full(m, -1, np.int64)
        ndep = np.zeros(m, np.int32)
        for j in range(m):
            for s in (cl[j], cr[j]):
                if s >= n:
                    ndep[j] += 1
                    parents[s - n] = j
        height = np.zeros(m, np.int64)
        for j in range(m - 1, -1, -1):
            p = parents[j]
            if p >= 0:
                height[j] = height[p] + 1
        done = np.full(m, 10**9, np.int64)
        remaining = ndep.copy()
        scheduled = 0
        r = 0
        while scheduled < m:
            ready = [j for j in range(m)
                     if remaining[j] == 0 and done[j] == 10**9
                     and all((s < n or done[s - n] <= r - dist) for s in (cl[j], cr[j]))]
            ready.sort(key=lambda j: (-height[j], j))
            for s_idx, j in enumerate(ready[:2]):
                rounds_of[b, j] = r
                slot_of[b, j] = s_idx
                done[j] = r
                scheduled += 1
                p = parents[j]
                if p >= 0:
                    remaining[p] -= 1
            r += 1
            assert r < 4 * m, "packer stuck"
        last_nonroot = max(last_nonroot, rounds_of[b, :m - 1].max())
        for s in (cl[m - 1], cr[m - 1]):
            if s >= n:
                root_child_max = max(root_child_max, int(rounds_of[b, s - n]))
    # root round: after every non-root merge AND >= 2 past every root child
    # so the root round's gather is prefetchable (non-sync)
    root_round = max(last_nonroot + 1, root_child_max + 2)
    rounds_of[:, m - 1] = root_round
    slot_of[:, m - 1] = 0
    R = root_round + 1
    sync_rounds = set()
    for b in range(B):
        for j in range(m):
            r = rounds_of[b, j]
            for s in (gl[b, j], gr[b, j]):
                if s >= n and rounds_of[b, s - n] > r - 2:
                    assert rounds_of[b, s - n] <= r - 1, "child not strictly earlier"
                    sync_rounds.add(int(r))
    return rounds_of, slot_of, R, sync_rounds


def _prepare(rep, fc1_w, fc1_b, fc2_w, fc2_b, rel_emb, n_per_bag, **kw):
    n_per_bag = int(n_per_bag)
    assert n_per_bag == NN and rep.shape[-1] == D
    rep = np.ascontiguousarray(rep, dtype=np.float32)

    w = _node_weight_like_reference(rep, n_per_bag)
    gl, gr = _huffman_schedule(w)
    rounds_of, slot_of, R, sync_rounds = _pack_rounds(gl, gr)
    SL = 64 + 2 * R
    zb = (not np.any(np.asarray(fc1_b))) and (not np.any(np.asarray(fc2_b)))
    use_fp8 = True

    # per-bag leaf renumbering so rounds 0/1 read fixed slots 0..7 (enables
    # plain direct DMAs for the first two gathers, no DGE latency)
    B = gl.shape[0]
    m = gl.shape[1]
    leaf_slot = np.tile(np.arange(NN, dtype=np.int64), (B, 1))
    direct01 = (0 not in sync_rounds) and (1 not in sync_rounds)
    if direct01:
        try:
            for b in range(B):
                want = []
                for r in range(2):
                    for s in range(2):
                        js = np.where((rounds_of[b] == r) & (slot_of[b] == s))[0]
                        assert len(js) == 1
                        j = js[0]
                        assert gl[b, j] < NN and gr[b, j] < NN
                        want += [gl[b, j], gr[b, j]]
                assert len(set(want)) == 8
                rest = [l for l in range(NN) if l not in set(want)]
                order = want + rest          # order[slot] = original leaf
                inv = np.empty(NN, np.int64)
                inv[np.array(order)] = np.arange(NN)
                leaf_slot[b] = inv
        except AssertionError:
            direct01 = False
            leaf_slot = np.tile(np.arange(NN, dtype=np.int64), (B, 1))

    nc = _build_program(R, SL, frozenset(sync_rounds), zb, use_fp8, direct01)

    import ml_dtypes
    mmdt = ml_dtypes.float8_e4m3fn if use_fp8 else ml_dtypes.bfloat16
    w1t = np.ascontiguousarray(np.asarray(fc1_w, np.float32).T).astype(mmdt)   # (2D, D)
    w2t = np.ascontiguousarray(np.asarray(fc2_w, np.float32).T).astype(mmdt)   # (D, D)
    b1b = np.ascontiguousarray(np.broadcast_to(np.asarray(fc1_b, np.float32), (128, D)))
    b2b = np.ascontiguousarray(np.broadcast_to(np.asarray(fc2_b, np.float32), (128, D)))
    ident = np.eye(128, dtype=ml_dtypes.bfloat16)

    merge_slot = 64 + 2 * rounds_of + slot_of          # (B, 63)

    in_maps = []
    for c in range(NCORES):
        b0 = c * NB
        gidx = np.zeros((128, 2 * R), np.int32)
        for s in range(2):
            for lb in range(NB):
                gidx[s * NB + lb, :] = lb * SL   # pads read a leaf row
        for lb in range(NB):
            b = b0 + lb
            for j in range(m):
                r = rounds_of[b, j]
                s = slot_of[b, j]
                ls, rs = gl[b, j], gr[b, j]
                ls = leaf_slot[b, ls] if ls < NN else merge_slot[b, ls - NN]
                rs = leaf_slot[b, rs] if rs < NN else merge_slot[b, rs - NN]
                gidx[s * NB + lb, 2 * r] = lb * SL + ls
                gidx[s * NB + lb, 2 * r + 1] = lb * SL + rs

        feats = np.zeros((NB * SL, D), ml_dtypes.bfloat16)
        leaves = np.tanh(rep[b0 * NN:(b0 + NB) * NN].reshape(NB, NN, D)).astype(ml_dtypes.bfloat16)
        f3h = feats.reshape(NB, SL, D)
        for lb in range(NB):
            f3h[lb, leaf_slot[b0 + lb], :] = leaves[lb]
        in_maps.append({
            "feats": feats,
            "w1t": w1t, "w2t": w2t,
            "b1b": b1b, "b2b": b2b, "gidx": gidx, "ident": ident,
        })
    return nc, in_maps


def kernel(rep, fc1_w, fc1_b, fc2_w, fc2_b, rel_emb, n_per_bag, **kw):
    nc, in_maps = _prepare(rep, fc1_w, fc1_b, fc2_w, fc2_b, rel_emb, n_per_bag)
    from concourse import bass_utils
    res = bass_utils.run_bass_kernel_spmd(nc, in_maps, core_ids=list(range(NCORES)))
    root = np.concatenate([res.results[c]["out"] for c in range(NCORES)], axis=0)
    scores = root.astype(np.float32) @ np.asarray(rel_emb, np.float32).T
    out = 1.0 / (1.0 + np.exp(-scores, dtype=np.float64))
    return np.ascontiguousarray(out.astype(np.float32))
